# revision 19
# baseline (speedup 1.0000x reference)
"""Additive (Bahdanau) attention Trainium2 kernel, SPMD over 8 NeuronCores.

Problem (full shapes): query/key/value (2, 8, 384, 64) f32, Wq/Wk (64, 64),
v_w (64,).  reference:
    qp = q @ Wq.T ; kp = k @ Wk.T
    score[b,h,t,s] = v_w . tanh(qp[b,h,t,:] + kp[b,h,s,:])
    attn = softmax(score, axis=-1) ; out = attn @ value
Returns (out, attn).

Sharding: 16 (b,h) pairs -> 2 per core, no cross-core comms.

Per-core algorithm (per pair):
  - projections on PE (qp natural (t, d); kp as kpT (d, s))
  - partitions of the big tanh tensor are interleaved p = 4*j + dd where
    j = t%32 and chunk c covers the strided d-set {16*dd + c}.  With this
    chunking the interleave is a pure reshape: qp_bias[p, 16g+c] =
    qp_flat[2048 g + 16 p + c] (one plain DMA via a DRAM bounce), and
    kp_rep rows 4j..4j+3 are just the whole kpT flat (32 plain DMAs).
    Score tile (128 t, 384 s) accumulates over 16 chunk-matmuls with
    lhsT = blockdiag-ish v tiles, using 4 PSUM col-strips (32 t's each).
  - DVE tensor_scalar_add broadcasts qp over s (bf16, 4x mode); ACT does
    one big tanh per chunk (128, 12*384); exp+row-sum via activation
    accum_out straight out of PSUM. Softmax skips max-subtraction
    (|score| <= sum|v| <= 8, exp is safe in f32).
  - attn@value: PE-transpose exp tiles, matmul against bf16 value,
    normalize by 1/denom per t at the end.
"""

import numpy as np
import ml_dtypes

B, H, T, S, D = 2, 8, 384, 384, 64
NCORES = 8
PAIRS = 2  # (b,h) pairs per core
NG = 12  # t-groups of 32 per pair
NC_CHUNKS = 16  # d-chunks of 4
BF16 = ml_dtypes.bfloat16

_cache = {}


def _build_program():
    import concourse.bass as bass
    import concourse.bacc as bacc
    import concourse.tile as tile
    import concourse.mybir as mybir
    from contextlib import ExitStack

    f32 = mybir.dt.float32
    bf16 = mybir.dt.bfloat16
    AF = mybir.ActivationFunctionType

    nc = bacc.Bacc(
        "TRN2",
        target_bir_lowering=False,
        debug=False,
        enable_asserts=False,
        num_devices=NCORES,
    )

    # inputs (per-core shards + replicated weights, host-formatted)
    qT_d = nc.dram_tensor("qT", [PAIRS, 64, 384], f32, kind="ExternalInput")
    kT_d = nc.dram_tensor("kT", [PAIRS, 64, 384], f32, kind="ExternalInput")
    val_d = nc.dram_tensor("valb", [PAIRS, 384, 64], bf16, kind="ExternalInput")
    wqT_d = nc.dram_tensor("wqT", [64, 64], f32, kind="ExternalInput")
    wkT_d = nc.dram_tensor("wkT", [64, 64], f32, kind="ExternalInput")
    vb_d = nc.dram_tensor("vblk", [128, 16 * 32], bf16, kind="ExternalInput")
    id_d = nc.dram_tensor("ident", [128, 128], f32, kind="ExternalInput")
    o_d = nc.dram_tensor("o", [PAIRS, 384, 64], f32, kind="ExternalOutput")
    aw_d = nc.dram_tensor("aw", [PAIRS, 384, 384], f32, kind="ExternalOutput")

    with tile.TileContext(nc) as tc:
        with ExitStack() as ctx:
            consts = ctx.enter_context(tc.tile_pool(name="consts", bufs=1))
            dram = ctx.enter_context(tc.tile_pool(name="dram", bufs=2, space="DRAM"))
            pp = ctx.enter_context(tc.tile_pool(name="pp", bufs=2))
            loop = ctx.enter_context(tc.tile_pool(name="loop", bufs=2))
            epi = ctx.enter_context(tc.tile_pool(name="epi", bufs=2))
            ps_score = ctx.enter_context(
                tc.tile_pool(name="ps_score", bufs=3, space="PSUM")
            )
            ps_misc = ctx.enter_context(
                tc.tile_pool(name="ps_misc", bufs=4, space="PSUM")
            )

            wq_s = consts.tile([64, 64], f32)
            nc.sync.dma_start(out=wq_s, in_=wqT_d.ap())
            wk_s = consts.tile([64, 64], f32)
            nc.sync.dma_start(out=wk_s, in_=wkT_d.ap())
            vb_s = consts.tile([128, 16 * 32], bf16)
            nc.sync.dma_start(out=vb_s, in_=vb_d.ap())
            id_s = consts.tile([128, 128], f32)
            nc.sync.dma_start(out=id_s, in_=id_d.ap())

            for p in range(PAIRS):
                # ---- load pair inputs
                qT_s = pp.tile([64, 384], f32, tag="qT")
                nc.sync.dma_start(out=qT_s, in_=qT_d.ap()[p])
                kT_s = pp.tile([64, 384], f32, tag="kT")
                nc.sync.dma_start(out=kT_s, in_=kT_d.ap()[p])
                val_s = pp.tile([128, 3 * 64], bf16, tag="val")
                for sc in range(3):
                    nc.sync.dma_start(
                        out=val_s[:, sc * 64 : (sc + 1) * 64],
                        in_=val_d.ap()[p, sc * 128 : (sc + 1) * 128, :],
                    )

                # ---- projections
                # qp natural (t, d): 3x  psum(128t, 64) = qT_slice.T @ wqT
                qp_dram = dram.tile([384, 64], f32, tag="qp_dram")
                for tck in range(3):
                    proj_ps = ps_misc.tile([128, 64], f32, tag="misc")
                    nc.tensor.matmul(
                        proj_ps,
                        lhsT=qT_s[:, tck * 128 : (tck + 1) * 128],
                        rhs=wq_s,
                        start=True,
                        stop=True,
                    )
                    qp_nat = pp.tile([128, 64], f32, tag="qp_nat")
                    nc.vector.tensor_copy(qp_nat, proj_ps)
                    nc.sync.dma_start(
                        out=qp_dram[tck * 128 : (tck + 1) * 128, :], in_=qp_nat
                    )
                # kpT (d, s) -> bf16, staged to DRAM
                proj_ps2 = ps_misc.tile([64, 384], f32, tag="misc")
                nc.tensor.matmul(proj_ps2, lhsT=wk_s, rhs=kT_s, start=True, stop=True)
                kpT_b = pp.tile([64, 384], bf16, tag="kpT")
                nc.vector.tensor_copy(kpT_b, proj_ps2)
                kp_dram = dram.tile([64, 384], bf16, tag="kp_dram")
                nc.sync.dma_start(out=kp_dram[:], in_=kpT_b)

                # ---- qp_bias[p, 16g+c] = qp_flat[2048g + 16p + c]: one DMA
                qp_bias = pp.tile([128, 192], f32, tag="qp_bias")
                in_ap = bass.AP(
                    tensor=qp_dram.tensor,
                    offset=qp_dram.offset,
                    ap=[[16, 128], [2048, 12], [1, 16]],
                )
                out_ap = bass.AP(
                    tensor=qp_bias.tensor,
                    offset=qp_bias.offset,
                    ap=[[192, 128], [16, 12], [1, 16]],
                )
                nc.sync.dma_start(out=out_ap, in_=in_ap)

                # ---- kp_rep[4j:4j+4, :] = whole kpT flat (strided chunks)
                kp_rep = pp.tile([128, 16 * 384], bf16, tag="kp_rep")
                for j in range(32):
                    in_ap = bass.AP(
                        tensor=kp_dram.tensor,
                        offset=kp_dram.offset,
                        ap=[[16 * 384, 4], [1, 16 * 384]],
                    )
                    nc.sync.dma_start(out=kp_rep[4 * j : 4 * j + 4, :], in_=in_ap)

                # ---- main loop: scores accumulate over chunks
                # padded to 512 f32 = exactly one 2KB PSUM bank per partition so
                # the 4 col-strip accumulation groups stay in disjoint zero regions
                score_ps = [
                    ps_score.tile([128, 512], f32, tag="score", name=f"score_p{p}_{i}")
                    for i in range(3)
                ]
                for c in range(NC_CHUNKS):
                    sum_buf = loop.tile([128, NG * 384], bf16, tag="sum")
                    for g in range(NG):
                        nc.vector.tensor_scalar_add(
                            sum_buf[:, g * 384 : (g + 1) * 384],
                            kp_rep[:, c * 384 : (c + 1) * 384],
                            qp_bias[:, 16 * g + c : 16 * g + c + 1],
                        )
                    tanh_buf = loop.tile([128, NG * 384], bf16, tag="tanh")
                    nc.scalar.activation(tanh_buf, sum_buf, AF.Tanh)
                    for sg in range(3):
                        for g2 in range(4):
                            g = 4 * sg + g2
                            nc.tensor.matmul(
                                score_ps[sg][32 * g2 : 32 * (g2 + 1), 0:384],
                                lhsT=vb_s[:, c * 32 : (c + 1) * 32],
                                rhs=tanh_buf[:, g * 384 : (g + 1) * 384],
                                start=(c == 0),
                                stop=(c == NC_CHUNKS - 1),
                                tile_position=(0, 32 * g2),
                                # the 4 col-strips interleave start/stop groups in
                                # partition-disjoint regions of one bank; the
                                # conservative group check can't express that
                                skip_group_check=True,
                            )

                # ---- softmax + outputs
                denom = epi.tile([128, 3], f32, tag="denom")
                exp_s = []
                for sg in range(3):
                    e = epi.tile([128, 384], f32, tag=f"exp{sg}", name=f"exp_p{p}_{sg}")
                    nc.scalar.activation(
                        e, score_ps[sg][:, 0:384], AF.Exp, accum_out=denom[:, sg : sg + 1]
                    )
                    exp_s.append(e)
                rden = epi.tile([128, 3], f32, tag="rden")
                nc.vector.reciprocal(rden, denom)

                attnT = epi.tile([128, 3 * 384], bf16, tag="attnT")
                for sg in range(3):
                    aw_f = epi.tile([128, 384], f32, tag="awf")
                    nc.vector.tensor_scalar_mul(
                        aw_f, exp_s[sg], rden[:, sg : sg + 1]
                    )
                    nc.sync.dma_start(
                        out=aw_d.ap()[p, sg * 128 : (sg + 1) * 128, :], in_=aw_f
                    )
                    for sc in range(3):
                        tr_ps = ps_misc.tile([128, 128], f32, tag="misc")
                        nc.tensor.transpose(
                            tr_ps,
                            in_=exp_s[sg][:, sc * 128 : (sc + 1) * 128],
                            identity=id_s,
                        )
                        nc.vector.tensor_copy(
                            attnT[:, sc * 384 + sg * 128 : sc * 384 + (sg + 1) * 128],
                            tr_ps,
                        )

                for tcki in range(3):
                    out_ps = ps_misc.tile([128, 64], f32, tag="misc")
                    for sc in range(3):
                        nc.tensor.matmul(
                            out_ps,
                            lhsT=attnT[:, sc * 384 + tcki * 128 : sc * 384 + (tcki + 1) * 128],
                            rhs=val_s[:, sc * 64 : (sc + 1) * 64],
                            start=(sc == 0),
                            stop=(sc == 2),
                        )
                    o_s = epi.tile([128, 64], f32, tag="os")
                    nc.vector.tensor_scalar_mul(o_s, out_ps, rden[:, tcki : tcki + 1])
                    nc.sync.dma_start(
                        out=o_d.ap()[p, tcki * 128 : (tcki + 1) * 128, :], in_=o_s
                    )

    nc.compile()
    return nc


def _host_inputs(query, key, value, Wq, Wk, v_w):
    """Per-core input dicts. Pure data formatting, no math."""
    qf = np.ascontiguousarray(query, np.float32).reshape(16, 384, 64)
    kf = np.ascontiguousarray(key, np.float32).reshape(16, 384, 64)
    vf = np.ascontiguousarray(value, np.float32).reshape(16, 384, 64)
    wqT = np.ascontiguousarray(np.asarray(Wq, np.float32).T)
    wkT = np.ascontiguousarray(np.asarray(Wk, np.float32).T)
    vw = np.asarray(v_w, np.float32)

    vblk = np.zeros((128, 16 * 32), np.float32)
    for c in range(16):
        for j in range(32):
            for dd in range(4):
                vblk[4 * j + dd, c * 32 + j] = vw[16 * dd + c]
    vblk = vblk.astype(BF16)
    ident = np.eye(128, dtype=np.float32)

    in_maps = []
    for i in range(NCORES):
        sl = slice(2 * i, 2 * i + 2)
        in_maps.append(
            {
                "qT": np.ascontiguousarray(qf[sl].transpose(0, 2, 1)),
                "kT": np.ascontiguousarray(kf[sl].transpose(0, 2, 1)),
                "valb": np.ascontiguousarray(vf[sl].astype(BF16)),
                "wqT": wqT,
                "wkT": wkT,
                "vblk": vblk,
                "ident": ident,
            }
        )
    return in_maps


def _assemble(results):
    out = np.stack([results[i]["o"] for i in range(NCORES)]).reshape(2, 8, 384, 64)
    aw = np.stack([results[i]["aw"] for i in range(NCORES)]).reshape(2, 8, 384, 384)
    return out.astype(np.float32), aw.astype(np.float32)


def kernel(query, key, value, Wq, Wk, v_w, _sim=False, _trace=False):
    in_maps = _host_inputs(query, key, value, Wq, Wk, v_w)

    if "nc" not in _cache:
        _cache["nc"] = _build_program()
    nc = _cache["nc"]

    if _sim:
        from concourse.bass_interp import CoreSim

        results = []
        for i in range(NCORES):
            sim = CoreSim(nc)
            for k, v in in_maps[i].items():
                sim.tensor(k)[:] = v
            sim.simulate(check_with_hw=False)
            results.append(
                {"o": np.array(sim.tensor("o")), "aw": np.array(sim.tensor("aw"))}
            )
        return _assemble(results)

    from concourse.bass_utils import run_bass_kernel_spmd

    res = run_bass_kernel_spmd(
        nc, in_maps, core_ids=list(range(NCORES)), trace=_trace
    )
    if _trace:
        _cache["last_result"] = res
    return _assemble(res.results)
